# revision 11
# baseline (speedup 1.0000x reference)
"""Conditioned temporal attention kernel for Trainium2 (Bass/Tile).

Computes, for each sequence n:
    h_proj  = gru_output @ W_h                      # (T, H)
    energy  = tanh(h_proj + condition @ W_c + bias) # (T, H)
    scores  = energy @ v                            # (T,)
    weights = softmax(scores)                       # (T,)
    context = weights @ gru_output                  # (H,)

Sharding: data-parallel over N across 8 NeuronCores (4 sequences each).

Per-core dataflow (all fp32, matmuls in float32r mode = full PE rate):
  - X = gru tile loaded naturally [t_part, h_free]; PE transpose-mode
    builds X^T tiles [h'_part, t_free] (PSUM), copied to SBUF (DVE/ACT).
  - GEMM: h_projT[h2, t] = sum_k W_h[k-tile][:, h2].T @ X^T[k-tile]
    accumulated over 8 k-tiles in PSUM; W stays SBUF-resident.
  - ACT computes tanh(psum + (c_proj+bias)[h2]) PSUM->SBUF; the bias add
    rides the activation's per-partition bias operand (transposed layout
    puts h2 on partitions, where c_proj/bias are constant along t).
  - scores[1, t] = v[h2-tile].T @ energyT accumulated over h2 tiles.
  - softmax on the [1, T] row (no max subtraction: |scores| < ~8, exp is
    safely in fp32 range, result is mathematically identical).
  - pass 2 reuses resident X^T: weights row is broadcast across
    partitions with a ones-vector matmul, then DVE tensor_tensor_reduce
    computes context^T[h] = sum_t X^T[h, t] * w[t] chunk by chunk.
"""

import numpy as np

import concourse.bass as bass
import concourse.bacc as bacc
import concourse.mybir as mybir
import concourse.tile as tile
from concourse.bass_utils import run_bass_kernel_spmd
from concourse.masks import make_identity

F32 = mybir.dt.float32
F32R = mybir.dt.float32r
N_CORES = 8
N, T, H, D = 32, 2048, 1024, 384
NL = N // N_CORES  # sequences per core


def build_nc(
    NL=NL,
    T=T,
    H=H,
    D=D,
    TC=512,            # t-chunk (moving free dim of the main GEMM)
    use_f32r=True,     # float32r matmul mode (1 cyc/row vs 4 for fp32)
    act_copy_every=4,  # every k-th X^T copy-back goes to ACT instead of DVE
):
    P = 128
    KH = H // P        # k tiles over h' (contraction)
    MH = H // P        # m tiles over h2 (output hidden)
    KD = (D + P - 1) // P
    assert D % P == 0
    NCH = T // TC      # chunks per sequence
    JT = TC // P       # 128-subtiles per chunk

    # float32r is required on BOTH matmul operands for the fast PE path;
    # walrus requires every producer of fp32r-consumed data to write
    # fp32r-typed (rounded) output, so the tiles themselves are typed
    # fp32r and DMA-loaded weights are staged through a DVE copy.
    mm_dt = F32R if use_f32r else F32

    def dve(ap):
        return ap.bitcast(F32) if use_f32r else ap

    nc = bacc.Bacc("TRN2", target_bir_lowering=False)
    gru = nc.declare_dram_parameter("gru_output", [NL, T, H], F32, isOutput=False)
    cond = nc.declare_dram_parameter("condition", [NL, D], F32, isOutput=False)
    wh = nc.declare_dram_parameter("W_h", [H, H], mm_dt, isOutput=False)
    wc = nc.declare_dram_parameter("W_c", [D, H], F32, isOutput=False)
    bias = nc.declare_dram_parameter("bias", [H], F32, isOutput=False)
    v = nc.declare_dram_parameter("v", [H], mm_dt, isOutput=False)
    out = nc.declare_dram_parameter("out", [NL, H], F32, isOutput=True)

    with tile.TileContext(nc) as tc:
        with (
            tc.tile_pool(name="consts", bufs=1) as consts,
            tc.tile_pool(name="natx", bufs=6) as natx_pool,
            tc.tile_pool(name="xt", bufs=KH) as xt_pool,
            tc.tile_pool(name="energy", bufs=3) as energy_pool,
            tc.tile_pool(name="scratch", bufs=2) as scratch_pool,
            tc.tile_pool(name="rows", bufs=2) as rows_pool,
            tc.tile_pool(name="acc", bufs=2 * KH + 2) as acc_pool,
            tc.tile_pool(name="ptp", bufs=2, space="PSUM") as ptp_pool,
            tc.tile_pool(name="ph", bufs=2, space="PSUM") as ph_pool,
            tc.tile_pool(name="ps", bufs=2, space="PSUM") as ps_pool,
            tc.tile_pool(name="pb", bufs=2, space="PSUM") as pb_pool,
        ):
            # ---- constants -------------------------------------------------
            wh_sb = []
            for k in range(KH):
                t_ = consts.tile([P, H], mm_dt, tag=f"wh{k}", name="wh_sb")
                nc.sync.dma_start(t_[:], wh[k * P:(k + 1) * P, :])
                wh_sb.append(t_)
            wc_sb = []
            condT = []
            for d in range(KD):
                t_ = consts.tile([P, H], F32, tag=f"wc{d}")
                nc.sync.dma_start(t_[:], wc[d * P:(d + 1) * P, :])
                wc_sb.append(t_)
                t2 = consts.tile([P, NL], F32, tag=f"condT{d}")
                nc.gpsimd.dma_start(
                    t2[:], cond[:, d * P:(d + 1) * P].transpose([1, 0])
                )
                condT.append(t2)
            bias_col = consts.tile([P, MH], F32, tag="bias_col")
            nc.gpsimd.dma_start(bias_col[:], bias.rearrange("(j p) -> p j", p=P))
            v_col = consts.tile([P, MH], mm_dt, tag="v_col")
            nc.gpsimd.dma_start(v_col[:], v.rearrange("(j p) -> p j", p=P))
            identity = consts.tile([P, P], F32, tag="identity")
            make_identity(nc, identity[:])
            ones_row = consts.tile([1, P], F32, tag="ones_row")
            nc.vector.memset(ones_row[:], 1.0)

            # ---- c_proj^T + bias: cb[m][h2_part, n] ------------------------
            cb = []
            for m in range(MH):
                pc = ph_pool.tile([P, TC], F32, tag="ph")
                for d in range(KD):
                    nc.tensor.matmul(
                        pc[:, :NL],
                        wc_sb[d][:, m * P:(m + 1) * P],
                        condT[d][:],
                        start=(d == 0),
                        stop=(d == KD - 1),
                    )
                cbm = consts.tile([P, NL], F32, tag=f"cb{m}")
                nc.vector.tensor_scalar_add(
                    cbm[:], pc[:, :NL], bias_col[:, m:m + 1]
                )
                cb.append(cbm)

            # ---- main loop over sequences ----------------------------------
            copy_i = 0
            for n in range(NL):
                xt = [
                    xt_pool.tile([P, T], mm_dt, tag="xt", name="xt") for _ in range(KH)
                ]
                scores_row = rows_pool.tile([1, T], F32, tag="scores")

                for c in range(NCH):
                    t0 = c * TC
                    # load natural X tiles and transpose into xt
                    for j in range(JT):
                        nx = natx_pool.tile([P, H], F32, tag="natx")
                        nc.sync.dma_start(
                            nx[:], gru[n, t0 + j * P:t0 + (j + 1) * P, :]
                        )
                        for i in range(KH):
                            ptp = ptp_pool.tile([P, P], F32, tag="ptp")
                            nc.tensor.transpose(
                                ptp[:], nx[:, i * P:(i + 1) * P], identity[:]
                            )
                            dst = xt[i][:, t0 + j * P:t0 + (j + 1) * P]
                            copy_i += 1
                            if copy_i % act_copy_every == 0:
                                nc.scalar.copy(dst, ptp[:])
                            else:
                                nc.vector.tensor_copy(dst, ptp[:])

                    # GEMM + tanh + scores for this chunk
                    ps = ps_pool.tile([1, TC], F32, tag="ps")
                    for m in range(MH):
                        ph = ph_pool.tile([P, TC], F32, tag="ph")
                        for k in range(KH):
                            nc.tensor.matmul(
                                ph[:],
                                wh_sb[k][:, m * P:(m + 1) * P],
                                xt[k][:, t0:t0 + TC],
                                start=(k == 0),
                                stop=(k == KH - 1),
                            )
                        en = energy_pool.tile([P, TC], mm_dt, tag="energy")
                        nc.scalar.activation(
                            en[:],
                            ph[:],
                            mybir.ActivationFunctionType.Tanh,
                            bias=cb[m][:, n:n + 1],
                        )
                        nc.tensor.matmul(
                            ps[:],
                            v_col[:, m:m + 1],
                            en[:],
                            start=(m == 0),
                            stop=(m == MH - 1),
                        )
                    nc.vector.tensor_copy(scores_row[:, t0:t0 + TC], ps[:])

                # ---- softmax over the [1, T] scores row --------------------
                w_row = rows_pool.tile([1, T], F32, tag="w_row")
                ssum = rows_pool.tile([1, 1], F32, tag="ssum")
                rec = rows_pool.tile([1, 1], F32, tag="rec")
                nc.scalar.activation(
                    w_row[:], scores_row[:], mybir.ActivationFunctionType.Exp
                )
                nc.vector.reduce_sum(
                    ssum[:], w_row[:], axis=mybir.AxisListType.X
                )
                nc.vector.reciprocal(rec[:], ssum[:])
                nc.vector.tensor_scalar_mul(w_row[:], w_row[:], rec[:])

                # ---- pass 2: context^T[h] = sum_t X^T[h, t] * w[t] ---------
                parts = [
                    acc_pool.tile([P, NCH], F32, tag="parts", name="parts")
                    for _ in range(KH)
                ]
                for c in range(NCH):
                    pb = pb_pool.tile([P, TC], F32, tag="pb")
                    nc.tensor.matmul(
                        pb[:],
                        ones_row[:],
                        w_row[:, c * TC:(c + 1) * TC],
                        start=True,
                        stop=True,
                    )
                    for i in range(KH):
                        scr = scratch_pool.tile([P, TC], F32, tag="scratch", name="scr")
                        nc.vector.scalar_tensor_tensor(
                            out=scr[:],
                            in0=dve(xt[i][:, c * TC:(c + 1) * TC]),
                            scalar=0.0,
                            in1=pb[:],
                            op0=mybir.AluOpType.bypass,
                            op1=mybir.AluOpType.mult,
                            accum_out=parts[i][:, c:c + 1],
                        )
                for i in range(KH):
                    ctx = acc_pool.tile([P, 1], F32, tag="ctx", name="ctx")
                    nc.vector.reduce_sum(
                        ctx[:], parts[i][:], axis=mybir.AxisListType.X
                    )
                    nc.sync.dma_start(
                        out[n, i * P:(i + 1) * P].unsqueeze(1), ctx[:]
                    )
    nc.compile()
    return nc


_CACHED = {}


def _get_nc():
    if "nc" not in _CACHED:
        _CACHED["nc"] = build_nc()
    return _CACHED["nc"]


def _run(inputs, trace=False, **spmd_kwargs):
    gru_output = np.ascontiguousarray(np.asarray(inputs["gru_output"], dtype=np.float32))
    condition = np.ascontiguousarray(np.asarray(inputs["condition"], dtype=np.float32))
    W_h = np.ascontiguousarray(np.asarray(inputs["W_h"], dtype=np.float32))
    W_c = np.ascontiguousarray(np.asarray(inputs["W_c"], dtype=np.float32))
    bias = np.ascontiguousarray(np.asarray(inputs["bias"], dtype=np.float32))
    v = np.ascontiguousarray(np.asarray(inputs["v"], dtype=np.float32))

    nc = _get_nc()
    core_ids = list(range(N_CORES))
    in_maps = []
    for i in core_ids:
        sl = slice(i * NL, (i + 1) * NL)
        in_maps.append(
            {
                "gru_output": gru_output[sl],
                "condition": condition[sl],
                "W_h": W_h,
                "W_c": W_c,
                "bias": bias,
                "v": v,
            }
        )
    res = run_bass_kernel_spmd(nc, in_maps, core_ids, trace=trace, **spmd_kwargs)
    out = np.concatenate([res.results[i]["out"] for i in core_ids], axis=0)
    return out, res


def kernel(**inputs) -> np.ndarray:
    return _run(inputs)[0]


if __name__ == "__main__":
    rng = np.random.default_rng(0)
    ins = {
        "gru_output": rng.standard_normal((N, T, H), dtype=np.float32),
        "condition": rng.standard_normal((N, D), dtype=np.float32),
        "W_h": rng.standard_normal((H, H), dtype=np.float32) / np.sqrt(H),
        "W_c": rng.standard_normal((D, H), dtype=np.float32) / np.sqrt(D),
        "bias": np.zeros(H, dtype=np.float32),
        "v": rng.standard_normal(H, dtype=np.float32) / np.sqrt(H),
    }
    out = kernel(**ins)
    print(out.shape, out.dtype, float(np.abs(out).mean()))


# revision 12
# speedup vs baseline: 1.0664x; 1.0664x over previous
"""Conditioned temporal attention kernel for Trainium2 (Bass/Tile).

Computes, for each sequence n:
    h_proj  = gru_output @ W_h                      # (T, H)
    energy  = tanh(h_proj + condition @ W_c + bias) # (T, H)
    scores  = energy @ v                            # (T,)
    weights = softmax(scores)                       # (T,)
    context = weights @ gru_output                  # (H,)

Sharding: data-parallel over N across 8 NeuronCores (4 sequences each).

Per-core dataflow (all fp32, matmuls in float32r mode = full PE rate):
  - X = gru tile loaded naturally [t_part, h_free]; PE transpose-mode
    builds X^T tiles [h'_part, t_free] (PSUM), copied to SBUF (DVE/ACT).
  - GEMM: h_projT[h2, t] = sum_k W_h[k-tile][:, h2].T @ X^T[k-tile]
    accumulated over 8 k-tiles in PSUM; W stays SBUF-resident.
  - ACT computes tanh(psum + (c_proj+bias)[h2]) PSUM->SBUF; the bias add
    rides the activation's per-partition bias operand (transposed layout
    puts h2 on partitions, where c_proj/bias are constant along t).
  - scores[1, t] = v[h2-tile].T @ energyT accumulated over h2 tiles.
  - softmax on the [1, T] row (no max subtraction: |scores| < ~8, exp is
    safely in fp32 range, result is mathematically identical).
  - pass 2 reuses resident X^T: weights row is broadcast across
    partitions with a ones-vector matmul, then DVE tensor_tensor_reduce
    computes context^T[h] = sum_t X^T[h, t] * w[t] chunk by chunk.
"""

import numpy as np

import concourse.bass as bass
import concourse.bacc as bacc
import concourse.mybir as mybir
import concourse.tile as tile
from concourse.bass_utils import run_bass_kernel_spmd
from concourse.masks import make_identity

F32 = mybir.dt.float32
F32R = mybir.dt.float32r
N_CORES = 8
N, T, H, D = 32, 2048, 1024, 384
NL = N // N_CORES  # sequences per core


def build_nc(
    NL=NL,
    T=T,
    H=H,
    D=D,
    TC=512,            # t-chunk (moving free dim of the main GEMM)
    use_f32r=True,     # float32r matmul mode (1 cyc/row vs 4 for fp32)
    act_copy_every=4,  # every k-th X^T copy-back goes to ACT instead of DVE
):
    P = 128
    KH = H // P        # k tiles over h' (contraction)
    MH = H // P        # m tiles over h2 (output hidden)
    KD = (D + P - 1) // P
    assert D % P == 0
    NCH = T // TC      # chunks per sequence
    JT = TC // P       # 128-subtiles per chunk

    # float32r is required on BOTH matmul operands for the fast PE path;
    # walrus requires every producer of fp32r-consumed data to write
    # fp32r-typed (rounded) output, so the tiles themselves are typed
    # fp32r and DMA-loaded weights are staged through a DVE copy.
    mm_dt = F32R if use_f32r else F32

    def dve(ap):
        return ap.bitcast(F32) if use_f32r else ap

    nc = bacc.Bacc("TRN2", target_bir_lowering=False)
    gru = nc.declare_dram_parameter("gru_output", [NL, T, H], F32, isOutput=False)
    cond = nc.declare_dram_parameter("condition", [NL, D], F32, isOutput=False)
    wh = nc.declare_dram_parameter("W_h", [H, H], mm_dt, isOutput=False)
    wc = nc.declare_dram_parameter("W_c", [D, H], F32, isOutput=False)
    bias = nc.declare_dram_parameter("bias", [H], F32, isOutput=False)
    v = nc.declare_dram_parameter("v", [H], mm_dt, isOutput=False)
    out = nc.declare_dram_parameter("out", [NL, H], F32, isOutput=True)

    with tile.TileContext(nc) as tc:
        with (
            tc.tile_pool(name="consts", bufs=1) as consts,
            tc.tile_pool(name="natx", bufs=8) as natx_pool,
            tc.tile_pool(name="xt", bufs=KH + 4) as xt_pool,
            tc.tile_pool(name="energy", bufs=3) as energy_pool,
            tc.tile_pool(name="scratch", bufs=2) as scratch_pool,
            tc.tile_pool(name="rows", bufs=2) as rows_pool,
            tc.tile_pool(name="acc", bufs=2 * KH + 2) as acc_pool,
            tc.tile_pool(name="ptp", bufs=2, space="PSUM") as ptp_pool,
            tc.tile_pool(name="ph", bufs=2, space="PSUM") as ph_pool,
            tc.tile_pool(name="ps", bufs=2, space="PSUM") as ps_pool,
            tc.tile_pool(name="pb", bufs=2, space="PSUM") as pb_pool,
        ):
            # ---- constants -------------------------------------------------
            wh_sb = []
            for k in range(KH):
                t_ = consts.tile([P, H], mm_dt, tag=f"wh{k}", name="wh_sb")
                nc.sync.dma_start(t_[:], wh[k * P:(k + 1) * P, :])
                wh_sb.append(t_)
            wc_sb = []
            condT = []
            for d in range(KD):
                t_ = consts.tile([P, H], F32, tag=f"wc{d}")
                nc.sync.dma_start(t_[:], wc[d * P:(d + 1) * P, :])
                wc_sb.append(t_)
                t2 = consts.tile([P, NL], F32, tag=f"condT{d}")
                nc.gpsimd.dma_start(
                    t2[:], cond[:, d * P:(d + 1) * P].transpose([1, 0])
                )
                condT.append(t2)
            bias_col = consts.tile([P, MH], F32, tag="bias_col")
            nc.gpsimd.dma_start(bias_col[:], bias.rearrange("(j p) -> p j", p=P))
            v_col = consts.tile([P, MH], mm_dt, tag="v_col")
            nc.gpsimd.dma_start(v_col[:], v.rearrange("(j p) -> p j", p=P))
            identity = consts.tile([P, P], F32, tag="identity")
            make_identity(nc, identity[:])
            ones_row = consts.tile([1, P], F32, tag="ones_row")
            nc.vector.memset(ones_row[:], 1.0)

            # ---- c_proj^T + bias: cb[m][h2_part, n] ------------------------
            cb = []
            for m in range(MH):
                pc = ph_pool.tile([P, TC], F32, tag="ph")
                for d in range(KD):
                    nc.tensor.matmul(
                        pc[:, :NL],
                        wc_sb[d][:, m * P:(m + 1) * P],
                        condT[d][:],
                        start=(d == 0),
                        stop=(d == KD - 1),
                    )
                cbm = consts.tile([P, NL], F32, tag=f"cb{m}")
                nc.vector.tensor_scalar_add(
                    cbm[:], pc[:, :NL], bias_col[:, m:m + 1]
                )
                cb.append(cbm)

            # ---- main loop over sequences ----------------------------------
            copy_i = 0
            for n in range(NL):
                # exp(scores) accumulates into w_row chunk by chunk; the
                # softmax normalization is folded into the final context
                # scale (context = (sum_t exp_t * x_t) / sum_t exp_t).
                w_row = rows_pool.tile([1, T], F32, tag="w_row", name="w_row")
                parts = [
                    acc_pool.tile([P, NCH], F32, tag="parts", name="parts")
                    for _ in range(KH)
                ]
                for c in range(NCH):
                    t0 = c * TC
                    # load natural X tiles and transpose into per-chunk xt
                    xt = [
                        xt_pool.tile([P, TC], mm_dt, tag="xt", name="xt")
                        for _ in range(KH)
                    ]
                    for j in range(JT):
                        nx = natx_pool.tile([P, H], F32, tag="natx", name="nx")
                        nc.sync.dma_start(
                            nx[:], gru[n, t0 + j * P:t0 + (j + 1) * P, :]
                        )
                        for i in range(KH):
                            ptp = ptp_pool.tile([P, P], F32, tag="ptp", name="ptp")
                            nc.tensor.transpose(
                                ptp[:], nx[:, i * P:(i + 1) * P], identity[:]
                            )
                            dst = xt[i][:, j * P:(j + 1) * P]
                            copy_i += 1
                            if copy_i % act_copy_every == 0:
                                nc.scalar.copy(dst, ptp[:])
                            else:
                                nc.vector.tensor_copy(dst, ptp[:])

                    # GEMM + tanh + scores for this chunk
                    ps = ps_pool.tile([1, TC], F32, tag="ps", name="ps")
                    for m in range(MH):
                        ph = ph_pool.tile([P, TC], F32, tag="ph", name="ph")
                        for k in range(KH):
                            nc.tensor.matmul(
                                ph[:],
                                wh_sb[k][:, m * P:(m + 1) * P],
                                xt[k][:],
                                start=(k == 0),
                                stop=(k == KH - 1),
                            )
                        en = energy_pool.tile([P, TC], mm_dt, tag="energy", name="en")
                        nc.scalar.activation(
                            en[:],
                            ph[:],
                            mybir.ActivationFunctionType.Tanh,
                            bias=cb[m][:, n:n + 1],
                        )
                        nc.tensor.matmul(
                            ps[:],
                            v_col[:, m:m + 1],
                            en[:],
                            start=(m == 0),
                            stop=(m == MH - 1),
                        )

                    # exp straight out of PSUM, then broadcast + weighted
                    # partial context for this chunk (unnormalized weights)
                    nc.scalar.activation(
                        w_row[:, t0:t0 + TC], ps[:],
                        mybir.ActivationFunctionType.Exp,
                    )
                    pb = pb_pool.tile([P, TC], F32, tag="pb", name="pb")
                    nc.tensor.matmul(
                        pb[:],
                        ones_row[:],
                        w_row[:, t0:t0 + TC],
                        start=True,
                        stop=True,
                    )
                    for i in range(KH):
                        scr = scratch_pool.tile([P, TC], F32, tag="scratch", name="scr")
                        nc.vector.scalar_tensor_tensor(
                            out=scr[:],
                            in0=dve(xt[i][:]),
                            scalar=0.0,
                            in1=pb[:],
                            op0=mybir.AluOpType.bypass,
                            op1=mybir.AluOpType.mult,
                            accum_out=parts[i][:, c:c + 1],
                        )

                # ---- normalize: context = parts_sum / sum(exp) -------------
                ssum = rows_pool.tile([1, 1], F32, tag="ssum", name="ssum")
                rec = rows_pool.tile([1, 1], F32, tag="rec", name="rec")
                nc.vector.reduce_sum(
                    ssum[:], w_row[:], axis=mybir.AxisListType.X
                )
                nc.vector.reciprocal(rec[:], ssum[:])
                prb = pb_pool.tile([P, TC], F32, tag="pb", name="prb")
                nc.tensor.matmul(
                    prb[:, 0:1], ones_row[:], rec[:], start=True, stop=True
                )
                rbc = acc_pool.tile([P, 1], F32, tag="rbc", name="rbc")
                nc.vector.tensor_copy(rbc[:], prb[:, 0:1])
                for i in range(KH):
                    ctx = acc_pool.tile([P, 1], F32, tag="ctx", name="ctx")
                    nc.vector.reduce_sum(
                        ctx[:], parts[i][:], axis=mybir.AxisListType.X
                    )
                    nc.vector.tensor_scalar_mul(ctx[:], ctx[:], rbc[:])
                    nc.sync.dma_start(
                        out[n, i * P:(i + 1) * P].unsqueeze(1), ctx[:]
                    )
    nc.compile()
    return nc


_CACHED = {}


def _get_nc():
    if "nc" not in _CACHED:
        _CACHED["nc"] = build_nc()
    return _CACHED["nc"]


def _run(inputs, trace=False, **spmd_kwargs):
    gru_output = np.ascontiguousarray(np.asarray(inputs["gru_output"], dtype=np.float32))
    condition = np.ascontiguousarray(np.asarray(inputs["condition"], dtype=np.float32))
    W_h = np.ascontiguousarray(np.asarray(inputs["W_h"], dtype=np.float32))
    W_c = np.ascontiguousarray(np.asarray(inputs["W_c"], dtype=np.float32))
    bias = np.ascontiguousarray(np.asarray(inputs["bias"], dtype=np.float32))
    v = np.ascontiguousarray(np.asarray(inputs["v"], dtype=np.float32))

    nc = _get_nc()
    core_ids = list(range(N_CORES))
    in_maps = []
    for i in core_ids:
        sl = slice(i * NL, (i + 1) * NL)
        in_maps.append(
            {
                "gru_output": gru_output[sl],
                "condition": condition[sl],
                "W_h": W_h,
                "W_c": W_c,
                "bias": bias,
                "v": v,
            }
        )
    res = run_bass_kernel_spmd(nc, in_maps, core_ids, trace=trace, **spmd_kwargs)
    out = np.concatenate([res.results[i]["out"] for i in core_ids], axis=0)
    return out, res


def kernel(**inputs) -> np.ndarray:
    return _run(inputs)[0]


if __name__ == "__main__":
    rng = np.random.default_rng(0)
    ins = {
        "gru_output": rng.standard_normal((N, T, H), dtype=np.float32),
        "condition": rng.standard_normal((N, D), dtype=np.float32),
        "W_h": rng.standard_normal((H, H), dtype=np.float32) / np.sqrt(H),
        "W_c": rng.standard_normal((D, H), dtype=np.float32) / np.sqrt(D),
        "bias": np.zeros(H, dtype=np.float32),
        "v": rng.standard_normal(H, dtype=np.float32) / np.sqrt(H),
    }
    out = kernel(**ins)
    print(out.shape, out.dtype, float(np.abs(out).mean()))
